# revision 1
# baseline (speedup 1.0000x reference)
"""CoAttention kernel for Trainium2 (8 NeuronCores, data-parallel over batch).

Math (per sample): ta = relu(seq_a @ W + b), tb likewise.  The reference
mean-pools the [N, rv_len, M] affinity before softmax, and mean-pooling
commutes with the dot product:

    atob_scores[n, l] = mean_m( ta[n,l,:] . tb_all_tokens[m,:] )
                      = ta[n,l,:] . mean_m( tb_all_tokens[m,:] )

so each side only needs a dot with the *other side's per-sample mean
feature vector* — the 52M-element affinity tensor is never materialized.

Per-core layout: host pre-transposes seq to [in_feat, tokens] so the FC
runs directly as PE matmuls with W stationary (fp32r, 1 col/cycle) and
taT [hdim, tokens] stays resident in SBUF.  Per-sample mean vectors come
free from the relu eviction's accum_out, so scores pipeline per sample
right behind the FC.  Scores are an M=1 matvec; softmax runs on a
[reviews, rv_len] view; the weighted sum broadcasts the softmax weights
across partitions (gpsimd) and does a segmented free-axis reduce (DVE).
"""
import sys

sys.path.insert(0, "/opt/trn_rl_repo")

import numpy as np

import concourse.bacc as bacc
import concourse.tile as tile
from concourse import mybir

# Problem shape (hardcoded per contest contract)
BZ, RV, RL, DIN, DH = 32, 10, 128, 300, 128
NCORES = 8
BPC = BZ // NCORES            # samples per core: 4
TPC = BPC * RV * RL           # tokens per core per side: 5120
TPS = RV * RL                 # tokens per sample: 1280
RPC = BPC * RV                # reviews per core: 40
NEG_INF = -1e9

f32 = mybir.dt.float32
f32r = mybir.dt.float32r
i32 = mybir.dt.int32
AF = mybir.ActivationFunctionType
AX = mybir.AxisListType

# d-chunks of the contraction dim (K <= 128)
DCH = [(0, 128), (128, 128), (256, 44)]
# free-dim chunks of one sample's tokens (N <= 512, >= 256 for fast fp32r)
NCH = [(0, 512), (512, 512), (1024, 256)]

_CACHE = {}


def _build(iters=1, serial=False, loop_n=0, stage=3):
    nc = bacc.Bacc("TRN2", target_bir_lowering=False, debug=False)

    sqt = {s: nc.dram_tensor(f"sqt_{s}", [DIN, TPC], f32r, kind="ExternalInput")
           for s in "ab"}
    msk2_d = nc.dram_tensor("msk2", [2 * RV, BPC * RL], i32,
                            kind="ExternalInput")
    w_d = nc.dram_tensor("w", [DIN, DH], f32r, kind="ExternalInput")
    bias_d = nc.dram_tensor("bias", [DH, 1], f32, kind="ExternalInput")
    ident_d = nc.dram_tensor("ident", [DH, DH], f32, kind="ExternalInput")

    out_v = {s: nc.dram_tensor(f"out_{s}", [RPC, DH], f32, kind="ExternalOutput")
             for s in "ab"}
    out_w = {s: nc.dram_tensor(f"outw_{s}", [RPC, RL], f32, kind="ExternalOutput")
             for s in "ab"}

    import contextlib
    outer_tc = tile.TileContext(nc) if not serial else None
    with (outer_tc if outer_tc is not None else contextlib.nullcontext()):
      for it_ in range(iters):
        pfx = f"i{it_}_" if iters > 1 else ""
        with (
            tile.TileContext(nc) if serial else contextlib.nullcontext()
        ) as maybe_tc:
          tc = maybe_tc if serial else outer_tc
          with (
            tc.For_i(0, loop_n, 1) if loop_n else contextlib.nullcontext()
          ):
           with (
            tc.tile_pool(name=pfx + "cst", bufs=1) as cst,
            tc.tile_pool(name=pfx + "seq", bufs=12) as seqp,
            tc.tile_pool(name=pfx + "big", bufs=1) as bigp,
            tc.tile_pool(name=pfx + "sm", bufs=2) as smp_pool,
            tc.tile_pool(name=pfx + "ps", bufs=2, space="PSUM") as ps,
        ):
            w_t = {}
            for c, (d0, dw) in enumerate(DCH):
                w_t[c] = cst.tile([dw, DH], f32r, tag=f"w{c}", name=f"{pfx}w_t{c}")
                nc.gpsimd.dma_start(w_t[c][:], w_d[d0:d0 + dw, :])
            bias_t = cst.tile([DH, 1], f32, tag="bias", name=pfx + "bias_t")
            nc.gpsimd.dma_start(bias_t[:], bias_d[:])
            # late-needed constants go on gpsimd so SP can stream seq tiles
            ident_t = cst.tile([DH, DH], f32, tag="ident", name=pfx + "ident_t")
            nc.gpsimd.dma_start(ident_t[:], ident_d[:])
            msk_t2 = cst.tile([2 * RV, BPC * RL], i32, tag="msk2", name=pfx + "msk_t2")
            nc.gpsimd.dma_start(msk_t2[:], msk2_d[:])

            taT, acc, mean, aoutT = {}, {}, {}, {}
            for s in "ab":
                taT[s] = bigp.tile([DH, TPC], f32r, tag=f"taT{s}",
                                   name=f"{pfx}taT_{s}")
                acc[s] = cst.tile([DH, BPC], f32, tag=f"acc{s}", name=f"{pfx}acc_{s}")
                mean[s] = cst.tile([DH, BPC], f32r, tag=f"mean{s}",
                                   name=f"{pfx}mean_{s}")
                aoutT[s] = cst.tile([DH, RPC], f32, tag=f"aoutT{s}",
                                    name=f"{pfx}aoutT_{s}")

            other = {"a": "b", "b": "a"}
            w2d_tiles = {}
            for smp in range(BPC):
                w2d_tiles[smp] = cst.tile(
                    [2 * RV, RL], f32, tag=f"w2d{smp}",
                    name=f"{pfx}w2ds_{smp}")

            def emit_fc_pair(smp):
                t0 = smp * TPS
                sq, pfc = {}, {}
                dma_eng = [nc.sync, nc.sync, nc.gpsimd]
                for s in ("b", "a"):
                    for c, (d0, dw) in enumerate(DCH):
                        sq[(s, c)] = seqp.tile([dw, TPS], f32r, tag="seq",
                                               name=f"{pfx}sq_{s}{smp}{c}")
                        dma_eng[c].dma_start(sq[(s, c)][:],
                                             sqt[s][d0:d0 + dw, t0:t0 + TPS])
                if stage < 1:
                    return
                for s in ("b", "a"):
                    pfc[s] = ps.tile([DH, TPS], f32, tag="fc", bufs=2,
                                     name=f"{pfx}pfc_{s}{smp}")
                # c-outer: 3 weight loads per sample pair instead of 18
                for c in range(3):
                    for s in ("b", "a"):
                        for n0, nw in NCH:
                            nc.tensor.matmul(
                                pfc[s][:, n0:n0 + nw],
                                w_t[c][:],
                                sq[(s, c)][:, n0:n0 + nw],
                                start=(c == 0), stop=(c == 2))
                for s in ("b", "a"):
                    nc.scalar.activation(
                        taT[s][:, t0:t0 + TPS], pfc[s][:], AF.Relu,
                        bias=bias_t[:], accum_out=acc[s][:, smp:smp + 1])
                    nc.scalar.mul(mean[s][:, smp:smp + 1],
                                  acc[s][:, smp:smp + 1], 1.0 / TPS)

            def emit_tail(smp):
                if stage < 2:
                    return
                t0 = smp * TPS
                # scores: M=1 matvec against the other side's mean, both sides
                psc = {}
                for s in ("a", "b"):
                    psc[s] = ps.tile([1, TPS], f32, tag="fc", bufs=2,
                                     name=f"{pfx}psc_{s}{smp}")
                    for n0, nw in NCH:
                        nc.tensor.matmul(
                            psc[s][:, n0:n0 + nw],
                            mean[other[s]][:, smp:smp + 1],
                            taT[s][:, t0 + n0:t0 + n0 + nw])
                scs = smp_pool.tile([2 * RV, RL], f32, tag="scs", bufs=3,
                                    name=f"{pfx}scs_{smp}")
                for i, s in enumerate(("a", "b")):
                    srow = smp_pool.tile([1, TPS], f32, tag="srow", bufs=4,
                                         name=f"{pfx}srow_{s}{smp}")
                    nc.scalar.copy(srow[:], psc[s][:])
                    nc.scalar.dma_start(scs[i * RV:(i + 1) * RV, :], srow[:])

                # masked softmax for both sides' reviews (a rows 0-9, b 10-19)
                lgs = smp_pool.tile([2 * RV, RL], f32, tag="lgs", bufs=3,
                                    name=f"{pfx}lgs_{smp}")
                nc.vector.memset(lgs[:], NEG_INF)
                nc.vector.copy_predicated(
                    lgs[:], msk_t2[:, smp * RL:(smp + 1) * RL], scs[:])
                negmax = smp_pool.tile([2 * RV, 1], f32, tag="negmax", bufs=3,
                                       name=f"{pfx}negmax_{smp}")
                nc.vector.reduce_max(out=negmax[:], in_=lgs[:],
                                     axis=AX.X, negate=True)
                e2d = smp_pool.tile([2 * RV, RL], f32, tag="e2d", bufs=3,
                                    name=f"{pfx}e2d_{smp}")
                ssum = smp_pool.tile([2 * RV, 1], f32, tag="ssum", bufs=3,
                                     name=f"{pfx}ssum_{smp}")
                nc.scalar.activation(e2d[:], lgs[:], AF.Exp, bias=negmax[:],
                                     accum_out=ssum[:])
                rec = smp_pool.tile([2 * RV, 1], f32, tag="rec", bufs=3,
                                    name=f"{pfx}rec_{smp}")
                nc.vector.reciprocal(rec[:], ssum[:])
                w2ds = w2d_tiles[smp]
                nc.vector.tensor_scalar_mul(w2ds[:], e2d[:], rec[:])

                # weighted sums
                for i, s in enumerate(("a", "b") if stage >= 3 else ()):
                    wflat = smp_pool.tile([1, TPS], f32, tag="wflat", bufs=4,
                                          name=f"{pfx}wflat_{s}{smp}")
                    nc.gpsimd.dma_start(
                        wflat[:], w2ds[i * RV:(i + 1) * RV, :])
                    wbc = smp_pool.tile([DH, TPS], f32, tag="wbc", bufs=3,
                                        name=f"{pfx}wbc_{s}{smp}")
                    nc.gpsimd.partition_broadcast(wbc[:], wflat[:])
                    tmp = smp_pool.tile([DH, TPS], f32, tag="tmp", bufs=3,
                                        name=f"{pfx}tmp_{s}{smp}")
                    nc.vector.tensor_tensor(
                        out=tmp[:], in0=taT[s][:, t0:t0 + TPS].bitcast(f32),
                        in1=wbc[:], op=mybir.AluOpType.mult)
                    nc.vector.reduce_sum(
                        out=aoutT[s][:, smp * RV:(smp + 1) * RV],
                        in_=tmp[:].rearrange("p (r l) -> p r l", r=RV),
                        axis=AX.X)

            # FC runs one sample ahead of the score/softmax/weighted-sum tail
            # so the in-order PE queue never stalls on an eviction.
            for smp in range(BPC):
                emit_fc_pair(smp)
                if smp >= 1:
                    emit_tail(smp - 1)
            emit_tail(BPC - 1)

            # ---- per-side epilogue: weights out, transpose, vectors out
            for si, s in enumerate(("a", "b") if stage >= 2 else ()):
                for smp in range(BPC):
                    nc.sync.dma_start(
                        out_w[s][smp * RV:(smp + 1) * RV, :],
                        w2d_tiles[smp][si * RV:(si + 1) * RV, :])
                ptp = ps.tile([RPC, DH], f32, tag="tp", bufs=2,
                              name=f"{pfx}ptp_{s}")
                nc.tensor.matmul(ptp[:], aoutT[s][:], ident_t[:],
                                 is_transpose=True)
                aout = smp_pool.tile([RPC, DH], f32, tag="aout",
                                     name=f"{pfx}aout_{s}")
                nc.vector.tensor_copy(aout[:], ptp[:])
                nc.sync.dma_start(out_v[s][:], aout[:])

    nc.compile()
    return nc


def build_in_maps(seq_a, seq_b, mask_a, mask_b, W, b):
    seq_a = np.asarray(seq_a, dtype=np.float32)
    seq_b = np.asarray(seq_b, dtype=np.float32)
    mask_a = np.asarray(mask_a, dtype=np.int32)
    mask_b = np.asarray(mask_b, dtype=np.int32)
    W = np.asarray(W, dtype=np.float32)
    b = np.asarray(b, dtype=np.float32)

    ident_np = np.eye(DH, dtype=np.float32)
    bias_np = np.ascontiguousarray(b.reshape(DH, 1))
    w_np = np.ascontiguousarray(W)

    in_maps = []
    for core in range(NCORES):
        b0 = core * BPC
        sl = {}
        for name, seq in (("a", seq_a), ("b", seq_b)):
            chunk = seq[b0:b0 + BPC].reshape(TPC, DIN)
            sl[f"sqt_{name}"] = np.ascontiguousarray(chunk.T)
        sl["msk2"] = np.ascontiguousarray(np.concatenate([
            mask[b0:b0 + BPC].reshape(BPC, RV, RL).transpose(1, 0, 2)
            .reshape(RV, BPC * RL) for mask in (mask_a, mask_b)], axis=0))
        sl["w"] = w_np
        sl["bias"] = bias_np
        sl["ident"] = ident_np
        in_maps.append(sl)
    return in_maps


def kernel(seq_a, seq_b, mask_a, mask_b, W, b):
    if "nc" not in _CACHE:
        _CACHE["nc"] = _build()
    nc = _CACHE["nc"]
    in_maps = build_in_maps(seq_a, seq_b, mask_a, mask_b, W, b)

    from concourse.bass_utils import run_bass_kernel_spmd
    res = run_bass_kernel_spmd(nc, in_maps, core_ids=list(range(NCORES)))

    a_out = np.concatenate([r["out_a"] for r in res.results], axis=0)
    b_out = np.concatenate([r["out_b"] for r in res.results], axis=0)
    atob_w = np.concatenate([r["outw_a"] for r in res.results], axis=0)
    btoa_w = np.concatenate([r["outw_b"] for r in res.results], axis=0)
    return (a_out, b_out, atob_w, btoa_w)



# revision 4
# speedup vs baseline: 1.3795x; 1.3795x over previous
"""CoAttention kernel for Trainium2 (8 NeuronCores, data-parallel over batch).

Math (per sample): ta = relu(seq_a @ W + b), tb likewise.  Mean-pooling the
[N, rv_len, M] affinity commutes with the dot product, so each side only
needs a dot with the other side's per-sample mean feature vector — the 52M
element affinity tensor is never materialized.

v2 design (fp16 on-chip, engines balanced against the ~17 us HBM floor):
 - Host casts seq/W to fp16 and pre-transposes to [300, tokens]; HBM traffic
   halves to ~6.2 MB/core.  PSUM accumulates fp32.
 - FC: c-outer over 3 K-chunks (128/128/44) shared across both sides per
   sample; PSUM windows of 512 (bank rule).  ACT evicts relu to fp16 taT and
   its accum_out gives the token-sum (-> per-sample mean) for free.
 - Scores: 10 accumulating matmuls per sample-side with a block-diagonal
   stationary ([128,10] zeros with the other side's mean in column j) land
   scores directly as PSUM [10,128] — one review per lane, so the whole
   masked softmax runs lane-parallel ([10,128] ops, not [1,1280]).
 - Mask: host precomputes (mask-1)*1e9; a single DVE add applies it.
 - exp on ACT emits fp16 e + fp32 row-sum; ACT scale-copy normalizes to
   fp16 weights.  A scalar-ring SBUF->SBUF DMA flattens [10,128] -> [1,1280]
   (separate HWDGE FIFO from the input stream), gpsimd partition_broadcast
   replicates it across partitions (bitcast to u32 to cut its per-element
   cost), and DVE does fp16 tensor_tensor mult + segmented reduce for the
   weighted sum.  Outputs are written transposed; host flips them back.
"""
import sys

sys.path.insert(0, "/opt/trn_rl_repo")

import numpy as np

import concourse.bacc as bacc
import concourse.tile as tile
from concourse import mybir

# Problem shape (hardcoded per contest contract)
BZ, RV, RL, DIN, DH = 32, 10, 128, 300, 128
NCORES = 8
BPC = BZ // NCORES            # samples per core: 4
TPS = RV * RL                 # tokens per sample-side: 1280
RPC = BPC * RV                # reviews per core: 40
NEG = -1e9

f32 = mybir.dt.float32
f16 = mybir.dt.float16
u32 = mybir.dt.uint32
AF = mybir.ActivationFunctionType
AX = mybir.AxisListType

DCH = [(0, 128), (128, 128), (256, 44)]       # K chunks of DIN=300
NW = [(0, 512), (512, 512), (1024, 256)]      # psum windows within 1280

_CACHE = {}


def _build():
    nc = bacc.Bacc("TRN2", target_bir_lowering=False, debug=False)

    c01 = {s: nc.dram_tensor(f"c01_{s}", [128, 2 * TPS * BPC], f16,
                             kind="ExternalInput") for s in "ab"}
    c2 = {s: nc.dram_tensor(f"c2_{s}", [44, TPS * BPC], f16,
                            kind="ExternalInput") for s in "ab"}
    w_d = nc.dram_tensor("w16", [128, 3 * DH], f16, kind="ExternalInput")
    bias_d = nc.dram_tensor("bias", [DH, 1], f32, kind="ExternalInput")
    mneg_d = nc.dram_tensor("mneg", [RV, 2 * BPC * RL], f32,
                            kind="ExternalInput")

    outv = {s: nc.dram_tensor(f"outv_{s}", [DH, RPC], f32,
                              kind="ExternalOutput") for s in "ab"}
    outw = {s: nc.dram_tensor(f"outw_{s}", [RPC, RL], f32,
                              kind="ExternalOutput") for s in "ab"}

    with tile.TileContext(nc) as tc:
        with (
            tc.tile_pool(name="cst", bufs=1) as cst,
            tc.tile_pool(name="seq", bufs=8) as seqp,
            tc.tile_pool(name="sm", bufs=3) as smpool,
            tc.tile_pool(name="wide", bufs=2) as widep,
            tc.tile_pool(name="ps", bufs=1, space="PSUM") as ps,
        ):
            # ---- constants (scalar/ACT HWDGE ring; sync ring carries seq)
            w16 = cst.tile([128, 3 * DH], f16, tag="w16")
            nc.scalar.dma_start(w16[:], w_d[:])
            bias_t = cst.tile([DH, 1], f32, tag="bias")
            nc.scalar.dma_start(bias_t[:], bias_d[:])
            mneg_t = cst.tile([RV, 2 * BPC * RL], f32, tag="mneg")
            nc.scalar.dma_start(mneg_t[:], mneg_d[:])

            taT, acc, mean16, aoutT, stat3 = {}, {}, {}, {}, {}
            for s in "ab":
                taT[s] = cst.tile([DH, BPC * TPS], f16, tag=f"taT{s}", name=f"taT_{s}")
                acc[s] = cst.tile([DH, BPC], f32, tag=f"acc{s}", name=f"acc_{s}")
                mean16[s] = cst.tile([DH, BPC], f16, tag=f"mean{s}", name=f"mean_{s}")
                aoutT[s] = cst.tile([DH, RPC], f32, tag=f"aoutT{s}", name=f"aoutT_{s}")
                stat3[s] = cst.tile([128, RV, 11], f16, tag=f"stat3{s}", name=f"stat3_{s}")
                nc.vector.memset(stat3[s][:], 0.0)

            # ---- input stream (sync HWDGE ring), sample-pipelined order
            c01_t, c2_t = {}, {}
            for s in "ab":
                c2_t[s] = cst.tile([44, TPS * BPC], f16, tag=f"c2{s}", name=f"c2t_{s}")
            for smp in range(BPC):
                for s in "ba":
                    c01_t[(s, smp)] = seqp.tile([128, 2 * TPS], f16,
                                                tag="c01",
                                                name=f"c01_{s}{smp}")
                    nc.sync.dma_start(
                        c01_t[(s, smp)][:],
                        c01[s][:, smp * 2 * TPS:(smp + 1) * 2 * TPS])
                if smp == 0:
                    for s in "ba":
                        nc.sync.dma_start(c2_t[s][:], c2[s][:])

            pfc = {s: ps.tile([128, 1536], f32, tag=f"pfc{s}", name=f"pfc_{s}")
                   for s in "ab"}

            other = {"a": "b", "b": "a"}

            def emit_fc(smp):
                t0 = smp * TPS
                # c-outer, sides interleaved: one stationary serves 6 windows
                for c, (d0, dw) in enumerate(DCH):
                    lhsT = w16[0:dw, c * DH:(c + 1) * DH]
                    for s in "ba":
                        for n0, nw in NW:
                            if c < 2:
                                mov = c01_t[(s, smp)][0:dw,
                                                      c * TPS + n0:
                                                      c * TPS + n0 + nw]
                            else:
                                mov = c2_t[s][:, t0 + n0:t0 + n0 + nw]
                            nc.tensor.matmul(pfc[s][:, n0:n0 + nw],
                                             lhsT, mov,
                                             start=(c == 0), stop=(c == 2))
                for s in "ba":
                    nc.scalar.activation(
                        taT[s][:, t0:t0 + TPS], pfc[s][:, 0:TPS], AF.Relu,
                        bias=bias_t[:], accum_out=acc[s][:, smp:smp + 1])
                    nc.scalar.mul(mean16[s][:, smp:smp + 1],
                                  acc[s][:, smp:smp + 1], 1.0 / TPS)

            def emit_tail(smp):
                t0 = smp * TPS
                for si, s in enumerate("ab"):
                    # stationary: zeros [128, 10*11]; mean(other) at col 0 of
                    # each 11-block => slice [10j, 10j+10) has it in col j
                    nc.vector.tensor_copy(
                        stat3[s][:, :, 0],
                        mean16[other[s]][:, smp:smp + 1]
                        .to_broadcast([128, RV]))
                    statf = stat3[s][:].rearrange("p a b -> p (a b)")
                    psc = ps.tile([RV, RL], f32, tag="psc", bufs=2,
                                  name=f"psc_{s}{smp}")
                    for j in range(RV):
                        nc.tensor.matmul(
                            psc[:, :], statf[:, 10 * j:10 * j + 10],
                            taT[s][:, t0 + j * RL:t0 + (j + 1) * RL],
                            start=(j == 0), stop=(j == RV - 1))
                    # masked softmax, lane-parallel [10, 128]
                    mcol = (smp * 2 + si) * RL
                    lgs = smpool.tile([RV, RL], f32, tag="lgs",
                                      name=f"lgs_{s}{smp}")
                    nc.vector.tensor_tensor(
                        out=lgs[:], in0=psc[:, :],
                        in1=mneg_t[:, mcol:mcol + RL],
                        op=mybir.AluOpType.add)
                    negmax = smpool.tile([RV, 1], f32, tag="negmax",
                                         name=f"nm_{s}{smp}")
                    nc.vector.reduce_max(out=negmax[:], in_=lgs[:],
                                         axis=AX.X, negate=True)
                    e16 = smpool.tile([RV, RL], f16, tag="e16",
                                      name=f"e16_{s}{smp}")
                    ssum = smpool.tile([RV, 1], f32, tag="ssum",
                                       name=f"ssum_{s}{smp}")
                    nc.scalar.activation(e16[:], lgs[:], AF.Exp,
                                         bias=negmax[:], accum_out=ssum[:])
                    rec = smpool.tile([RV, 1], f32, tag="rec",
                                      name=f"rec_{s}{smp}")
                    nc.vector.reciprocal(rec[:], ssum[:])
                    w2d16 = smpool.tile([RV, RL], f16, tag="w2d16",
                                        name=f"w2d_{s}{smp}")
                    nc.scalar.activation(w2d16[:], e16[:], AF.Copy,
                                         scale=rec[:])
                    # weights out (SWDGE casts fp16 -> fp32 during DMA)
                    nc.gpsimd.dma_start(
                        outw[s][smp * RV:(smp + 1) * RV, :], w2d16[:])
                    # flatten on the scalar HWDGE ring (not the seq ring)
                    erow = smpool.tile([1, TPS], f16, tag="erow",
                                       name=f"erow_{s}{smp}")
                    nc.scalar.dma_start(erow[:], w2d16[:])
                    wbc = widep.tile([128, TPS], f16, tag="wbc",
                                     name=f"wbc_{s}{smp}")
                    nc.gpsimd.partition_broadcast(
                        wbc[:].bitcast(u32), erow[:].bitcast(u32))
                    tmp16 = widep.tile([128, TPS], f16, tag="tmp",
                                       name=f"tmp_{s}{smp}")
                    nc.vector.tensor_tensor(
                        out=tmp16[:], in0=taT[s][:, t0:t0 + TPS],
                        in1=wbc[:], op=mybir.AluOpType.mult)
                    nc.vector.reduce_sum(
                        out=aoutT[s][:, smp * RV:(smp + 1) * RV],
                        in_=tmp16[:].rearrange("p (r l) -> p r l", r=RV),
                        axis=AX.X)

            # FC one sample ahead of the tail so the in-order PE queue
            # never stalls on an eviction.
            for smp in range(BPC):
                emit_fc(smp)
                if smp >= 1:
                    emit_tail(smp - 1)
            emit_tail(BPC - 1)

            for s in "ab":
                nc.scalar.dma_start(outv[s][:], aoutT[s][:])

    nc.compile()
    return nc


def build_in_maps(seq_a, seq_b, mask_a, mask_b, W, b):
    seq_a = np.asarray(seq_a, dtype=np.float32)
    seq_b = np.asarray(seq_b, dtype=np.float32)
    mask_a = np.asarray(mask_a, dtype=np.int32)
    mask_b = np.asarray(mask_b, dtype=np.int32)
    W = np.asarray(W, dtype=np.float32)
    b = np.asarray(b, dtype=np.float32)

    # W chunks as [dw, 128] packed into [128, 384]
    w16 = np.zeros((128, 3 * DH), np.float16)
    for c, (d0, dw) in enumerate(DCH):
        w16[0:dw, c * DH:(c + 1) * DH] = W[d0:d0 + dw, :].astype(np.float16)
    bias_np = np.ascontiguousarray(b.reshape(DH, 1))

    in_maps = []
    for core in range(NCORES):
        b0 = core * BPC
        sl = {"w16": w16, "bias": bias_np}
        for name, seq in (("a", seq_a), ("b", seq_b)):
            chunk = seq[b0:b0 + BPC].reshape(BPC * TPS, DIN)
            t16 = np.ascontiguousarray(chunk.T).astype(np.float16)
            c01 = np.empty((128, 2 * TPS * BPC), np.float16)
            for smp in range(BPC):
                o = smp * 2 * TPS
                c01[:, o:o + TPS] = t16[0:128, smp * TPS:(smp + 1) * TPS]
                c01[:, o + TPS:o + 2 * TPS] = \
                    t16[128:256, smp * TPS:(smp + 1) * TPS]
            sl[f"c01_{name}"] = c01
            sl[f"c2_{name}"] = np.ascontiguousarray(t16[256:300, :])
        # additive mask rows: review r on lane r; col block (smp, side)
        mneg = np.empty((RV, 2 * BPC * RL), np.float32)
        for smp in range(BPC):
            for si, mask in enumerate((mask_a, mask_b)):
                m = mask[b0 + smp].astype(np.float32)  # [RV, RL]
                mneg[:, (smp * 2 + si) * RL:(smp * 2 + si + 1) * RL] = \
                    (m - 1.0) * 1e9
        sl["mneg"] = mneg
        in_maps.append(sl)
    return in_maps


def kernel(seq_a, seq_b, mask_a, mask_b, W, b):
    if "nc" not in _CACHE:
        _CACHE["nc"] = _build()
    nc = _CACHE["nc"]
    in_maps = build_in_maps(seq_a, seq_b, mask_a, mask_b, W, b)

    from concourse.bass_utils import run_bass_kernel_spmd
    res = run_bass_kernel_spmd(nc, in_maps, core_ids=list(range(NCORES)))

    a_out = np.concatenate([r["outv_a"].T for r in res.results], axis=0)
    b_out = np.concatenate([r["outv_b"].T for r in res.results], axis=0)
    atob_w = np.concatenate([r["outw_a"] for r in res.results], axis=0)
    btoa_w = np.concatenate([r["outw_b"] for r in res.results], axis=0)
    return (a_out, b_out, atob_w, btoa_w)
